# revision 9
# baseline (speedup 1.0000x reference)
"""Dense 3-layer GAT on 8 TRN2 NeuronCores.

Sharding: each core owns 512 query nodes (rows of the attention score
matrix). Per layer, each core computes h = x @ W and f = x @ (W @ a) for
its own nodes, AllGathers h (bf16) and f (f32) across the 8 cores, then
computes its 512-query slab of masked-softmax attention and the attended
output.

Everything on device is kept transposed (features on partitions, nodes
on the free dim) so layer outputs feed the next layer's matmuls with no
transposes. The h/attention path runs in bf16 (TensorE streams bf16 at
2x the fp32 rate and DVE hits its packed modes); the attention-logit
path (f = x @ (W@a), leaky-relu bias, exp input) stays fp32 since the
softmax is sensitive to absolute errors in the logits.

kernel(**inputs) takes the full unsharded inputs and returns the full
[4096, 256] output.
"""

from contextlib import ExitStack

import numpy as np
import ml_dtypes

import concourse.mybir as mybir
import concourse.tile as tile
from concourse import bacc
from concourse.bass_utils import run_bass_kernel_spmd

P = 128
N_NODES = 4096
S = 512                    # nodes per core
NB = N_NODES // P          # 32 global key blocks
H = 4
LAYERS = [(512, 512), (2048, 512), (2048, 64)]
F32 = mybir.dt.float32
BF16 = mybir.dt.bfloat16
AF = mybir.ActivationFunctionType
ALU = mybir.AluOpType

_CACHE = {}


def _build():
    nc = bacc.Bacc("TRN2", target_bir_lowering=False, debug=False, num_devices=8)

    xT0_d = nc.dram_tensor("xT0", [512, S], F32, kind="ExternalInput")
    adjT_d = nc.dram_tensor("adjT", [N_NODES, S], BF16, kind="ExternalInput")
    W_d = []
    WA_d = []
    for li, (fin, fout) in enumerate(LAYERS):
        W_d.append(nc.dram_tensor(f"W{li}", [H, fin, fout], BF16, kind="ExternalInput"))
        WA_d.append(nc.dram_tensor(f"WA{li}", [fin, 2 * H], F32, kind="ExternalInput"))
    outT_d = nc.dram_tensor("outT", [H * 64, S], F32, kind="ExternalOutput")

    with tile.TileContext(nc) as tc:
        with ExitStack() as ctx:
            constp = ctx.enter_context(tc.tile_pool(name="const", bufs=1))
            adjp = ctx.enter_context(tc.tile_pool(name="adjp", bufs=1))
            xtp = ctx.enter_context(tc.tile_pool(name="xt", bufs=20))
            xbp = ctx.enter_context(tc.tile_pool(name="xb", bufs=21))
            wtp = ctx.enter_context(tc.tile_pool(name="wt", bufs=17))
            wap = ctx.enter_context(tc.tile_pool(name="wap", bufs=17))
            hfp = ctx.enter_context(tc.tile_pool(name="hfp", bufs=3))
            hgp = ctx.enter_context(tc.tile_pool(name="hgp", bufs=4))
            scp = ctx.enter_context(tc.tile_pool(name="scp", bufs=2))
            fbp = ctx.enter_context(tc.tile_pool(name="fbp", bufs=1))
            fdp = ctx.enter_context(tc.tile_pool(name="fdp", bufs=2))
            frp = ctx.enter_context(tc.tile_pool(name="frp", bufs=3))
            evp = ctx.enter_context(tc.tile_pool(name="evp", bufs=3))
            rcp = ctx.enter_context(tc.tile_pool(name="rcp", bufs=2))
            psA = ctx.enter_context(tc.tile_pool(name="psA", bufs=2, space="PSUM"))
            psO = ctx.enter_context(tc.tile_pool(name="psO", bufs=4, space="PSUM"))
            psR = ctx.enter_context(tc.tile_pool(name="psR", bufs=1, space="PSUM"))
            dr = ctx.enter_context(tc.tile_pool(name="dram", bufs=1, space="DRAM"))

            ones_r = constp.tile([1, P], F32, tag="ones_r")
            nc.any.memset(ones_r[:], 1.0)
            ones_c = constp.tile([P, 1], BF16, tag="ones_c")
            nc.any.memset(ones_c[:], 1.0)

            # resident adjacency (transposed slab), bf16, [key m, own query n]
            adjT_res = adjp.tile([P, NB, S], BF16, tag="adjT")
            nc.sync.dma_start(
                adjT_res[:], adjT_d[:].rearrange("(nb p) n -> p nb n", p=P)
            )

            # layer-0 x^T (own nodes): f32 for the f matmuls + bf16 for h
            xt_cur = []
            xb_cur = []
            for kb in range(4):
                t = xtp.tile([P, S], F32, tag="xt")
                nc.sync.dma_start(t[:], xT0_d[kb * P:(kb + 1) * P, :])
                xt_cur.append(t)
                tb = xbp.tile([P, S], BF16, tag="xb")
                nc.vector.tensor_copy(tb[:], t[:])
                xb_cur.append(tb)

            for li, (fin, fout) in enumerate(LAYERS):
                KB = fin // P
                agh_in = dr.tile([H, S, fout], BF16, tag=f"aghi{li}")
                agh_out = dr.tile([8, H, S, fout], BF16, tag=f"agho{li}",
                                  addr_space="Shared")
                agf_in = dr.tile([2 * H, S], F32, tag=f"agfi{li}")
                agf_out = dr.tile([8, 2 * H, S], F32, tag=f"agfo{li}",
                                  addr_space="Shared")

                # ---- phase A1: f = x @ WA (fp32) ----
                wa_tiles = []
                for kb in range(KB):
                    t = wap.tile([P, 2 * H], F32, tag="wa")
                    nc.sync.dma_start(t[:], WA_d[li][kb * P:(kb + 1) * P, :])
                    wa_tiles.append(t)
                for b in range(4):
                    pf = psA.tile([P, S], F32, tag="ph", name=f"pf{li}_{b}")
                    for kb in range(KB):
                        nc.tensor.matmul(
                            pf[:, 0:2 * H],
                            xt_cur[kb][:, b * P:(b + 1) * P],
                            wa_tiles[kb][:],
                            start=(kb == 0), stop=(kb == KB - 1),
                        )
                    f_sb = hfp.tile([P, 2 * H], F32, tag="fsb")
                    nc.vector.tensor_copy(f_sb[:], pf[:, 0:2 * H])
                    for j in range(2 * H):
                        nc.sync.dma_start(
                            agf_in[j, b * P:(b + 1) * P], f_sb[:, j:j + 1]
                        )

                # ---- phase A2: h = x @ W (bf16), shared-weight loop ----
                w_tiles = {}
                for h in range(H):
                    for kb in range(KB):
                        t = wtp.tile([P, fout], BF16, tag="wt", name=f"w{li}_{h}_{kb}")
                        nc.sync.dma_start(t[:], W_d[li][h, kb * P:(kb + 1) * P, :])
                        w_tiles[(h, kb)] = t
                for h in range(H):
                    for b in range(4):
                        ph = psA.tile([P, S], F32, tag="ph", name=f"ph{li}_{h}_{b}")
                        for kb in range(KB):
                            nc.tensor.matmul(
                                ph[:, 0:fout],
                                xb_cur[kb][:, b * P:(b + 1) * P],
                                w_tiles[(h, kb)][:],
                                start=(kb == 0), stop=(kb == KB - 1),
                            )
                        h_sb = hfp.tile([P, fout], BF16, tag="hsb")
                        nc.vector.tensor_copy(h_sb[:], ph[:, 0:fout])
                        nc.gpsimd.dma_start(agh_in[h, b * P:(b + 1) * P, :], h_sb[:])

                # ---- phase B: AllGather h (bf16) and f (f32) ----
                nc.gpsimd.collective_compute(
                    "AllGather", ALU.bypass,
                    replica_groups=[list(range(8))],
                    ins=[agf_in[:].opt()], outs=[agf_out[:].opt()],
                )
                nc.gpsimd.collective_compute(
                    "AllGather", ALU.bypass,
                    replica_groups=[list(range(8))],
                    ins=[agh_in[:].opt()], outs=[agh_out[:].opt()],
                )

                # ---- phase C: f_src broadcast + f_dst layout ----
                fsb_bcast = fbp.tile([P, H, S], F32, tag="fsb_b")
                for h in range(H):
                    fr = frp.tile([1, S], F32, tag="fr")
                    nc.sync.dma_start(fr[:], agf_in[2 * h:2 * h + 1, :])
                    pb = psA.tile([P, S], F32, tag="ph", name=f"pb{li}_{h}")
                    nc.tensor.matmul(pb[:], ones_r[:], fr[:], start=True, stop=True)
                    nc.scalar.copy(fsb_bcast[:, h, :], pb[:])
                fdst_sb = fdp.tile([P, H, NB], F32, tag="fdst")
                for h in range(H):
                    for r in range(8):
                        nc.sync.dma_start(
                            fdst_sb[:, h, r * 4:(r + 1) * 4],
                            agf_out[r, 2 * h + 1, :].rearrange("(mh ml) -> ml mh", ml=P),
                        )

                # ---- phase D: attention ----
                xt_next = []
                xb_next = []
                nob = 1 if fout == 64 else 4
                for h in range(H):
                    po = [
                        psO.tile([P, S], F32, tag="po", name=f"po{li}_{h}_{ob}")
                        for ob in range(nob)
                    ]
                    prs = psR.tile([1, S], F32, tag="prs", name=f"prs{li}_{h}")
                    for mbg in range(NB // 4):
                        lr4 = scp.tile([P, 4, S], F32, tag="lr")
                        for i in range(4):
                            nc.scalar.activation(
                                lr4[:, i, :], fsb_bcast[:, h, :], AF.Prelu,
                                bias=fdst_sb[:, h, 4 * mbg + i:4 * mbg + i + 1],
                                scale=1.0, alpha=0.2,
                            )
                        ex4 = scp.tile([P, 4, S], BF16, tag="ex")
                        nc.scalar.activation(ex4[:], lr4[:], AF.Exp, bias=0.0, scale=1.0)
                        st4 = scp.tile([P, 4, S], BF16, tag="st")
                        nc.vector.tensor_tensor(
                            st4[:], ex4[:], adjT_res[:, 4 * mbg:4 * mbg + 4, :], ALU.mult
                        )
                        for i in range(4):
                            mb = 4 * mbg + i
                            r, bsub = mb // 4, mb % 4
                            s_t = st4[:, i, :]
                            hg = hgp.tile([P, fout], BF16, tag="hg")
                            nc.gpsimd.dma_start(
                                hg[:], agh_out[r, h, bsub * P:(bsub + 1) * P, :]
                            )
                            for ob in range(nob):
                                nc.tensor.matmul(
                                    po[ob][:, :] if fout != 64 else po[ob][0:64, :],
                                    hg[:, ob * P:(ob + 1) * P] if fout != 64 else hg[:],
                                    s_t,
                                    start=(mb == 0), stop=(mb == NB - 1),
                                )
                            nc.tensor.matmul(
                                prs[:], ones_c[:], s_t,
                                start=(mb == 0), stop=(mb == NB - 1),
                            )

                    # normalize + elu -> next layer's x^T tiles
                    rsum = rcp.tile([1, S], F32, tag="rsum")
                    nc.vector.tensor_copy(rsum[:], prs[:])
                    pb2 = psA.tile([P, S], F32, tag="ph", name=f"pb2{li}_{h}")
                    nc.tensor.matmul(pb2[:], ones_r[:], rsum[:], start=True, stop=True)
                    rb0 = rcp.tile([P, S], F32, tag="rb0")
                    nc.scalar.copy(rb0[:], pb2[:])
                    rb = rcp.tile([P, S], F32, tag="rb")
                    nc.vector.reciprocal(rb[:], rb0[:])

                    rows = 64 if fout == 64 else P
                    for ob in range(nob):
                        src = po[ob][0:64, :] if fout == 64 else po[ob][:]
                        t0 = evp.tile([rows, S], F32, tag="t0")
                        nc.vector.tensor_tensor(t0[:], src, rb[0:rows, :], ALU.mult)
                        # elu(x) = min(exp(x) - 1, relu(x))
                        em = evp.tile([rows, S], F32, tag="em")
                        nc.scalar.activation(em[:], t0[:], AF.Exp, bias=0.0, scale=1.0)
                        rl = evp.tile([rows, S], F32, tag="rl")
                        nc.vector.tensor_scalar_max(rl[:], t0[:], 0.0)
                        xnt = xtp.tile([rows, S], F32, tag="xt")
                        nc.vector.scalar_tensor_tensor(
                            xnt[:], em[:], -1.0, rl[:], ALU.add, ALU.min
                        )
                        if li == 2:
                            # final jax.nn.elu on top of the per-layer elu
                            em2 = evp.tile([rows, S], F32, tag="em")
                            nc.scalar.activation(em2[:], xnt[:], AF.Exp, bias=0.0, scale=1.0)
                            rl2 = evp.tile([rows, S], F32, tag="rl")
                            nc.vector.tensor_scalar_max(rl2[:], xnt[:], 0.0)
                            x2 = xtp.tile([rows, S], F32, tag="xt")
                            nc.vector.scalar_tensor_tensor(
                                x2[:], em2[:], -1.0, rl2[:], ALU.add, ALU.min
                            )
                            xnt = x2
                        xt_next.append(xnt)
                        if li < 2:
                            xbn = xbp.tile([rows, S], BF16, tag="xb")
                            nc.vector.tensor_copy(xbn[:], xnt[:])
                            xb_next.append(xbn)

                xt_cur = xt_next
                xb_cur = xb_next

            # final output: xt_cur is 4 tiles of [64, 512] (head-major)
            for h in range(H):
                nc.sync.dma_start(outT_d[h * 64:(h + 1) * 64, :], xt_cur[h][:])

    nc.compile()
    return nc


def build_in_maps(inputs):
    node_feats = np.ascontiguousarray(inputs["node_feats"], dtype=np.float32)
    adj = np.asarray(inputs["adj"], dtype=np.float32)
    Ws = [np.asarray(inputs[f"W{i}"], dtype=np.float32) for i in range(3)]
    As = [np.asarray(inputs[f"a{i}"], dtype=np.float32) for i in range(3)]

    WAs = []
    for W, a in zip(Ws, As):
        wa = np.einsum(
            "hfo,hjo->fhj", W.astype(np.float64), a.astype(np.float64)
        ).reshape(W.shape[1], 2 * H).astype(np.float32)
        WAs.append(np.ascontiguousarray(wa))
    Wbf = [W.astype(ml_dtypes.bfloat16) for W in Ws]

    in_maps = []
    for c in range(8):
        rows = slice(c * S, (c + 1) * S)
        m = {
            "xT0": np.ascontiguousarray(node_feats[rows].T),
            "adjT": np.ascontiguousarray(adj[rows].T).astype(ml_dtypes.bfloat16),
        }
        for i in range(3):
            m[f"W{i}"] = Wbf[i]
            m[f"WA{i}"] = WAs[i]
        in_maps.append(m)
    return in_maps


def kernel(**inputs):
    if "nc" not in _CACHE:
        _CACHE["nc"] = _build()
    nc = _CACHE["nc"]
    in_maps = build_in_maps(inputs)
    res = run_bass_kernel_spmd(nc, in_maps, core_ids=list(range(8)))
    out = np.concatenate([r["outT"].T for r in res.results], axis=0)
    return np.ascontiguousarray(out, dtype=np.float32)


if __name__ == "__main__":
    rng = np.random.default_rng(0)
    fake = {
        "node_feats": rng.standard_normal((N_NODES, 512), dtype=np.float32),
        "edge_feats": rng.standard_normal((131072, 16), dtype=np.float32),
        "edge_indices": rng.integers(0, N_NODES, (2, 131072)).astype(np.int32),
        "adj": np.maximum(
            (rng.random((N_NODES, N_NODES)) < 0.01).astype(np.float32),
            np.eye(N_NODES, dtype=np.float32),
        ),
    }
    for i, (fin, fout) in enumerate(LAYERS):
        fake[f"W{i}"] = (rng.standard_normal((H, fin, fout)) * 0.05).astype(np.float32)
        fake[f"a{i}"] = (rng.standard_normal((H, 2, fout)) * 0.05).astype(np.float32)
    o = kernel(**fake)
    print("kernel output", o.shape, o.dtype, np.abs(o).mean())


# revision 11
# speedup vs baseline: 1.0114x; 1.0114x over previous
"""Dense 3-layer GAT on 8 TRN2 NeuronCores.

Sharding: each core owns 512 query nodes (rows of the attention score
matrix). Per layer, each core computes h = x @ W and f = x @ (W @ a) for
its own nodes, AllGathers h (bf16) and f (f32) across the 8 cores, then
computes its 512-query slab of masked-softmax attention and the attended
output.

Everything on device is kept transposed (features on partitions, nodes
on the free dim) so layer outputs feed the next layer's matmuls with no
transposes. The h/attention path runs in bf16 (TensorE streams bf16 at
2x the fp32 rate and DVE hits its packed modes); the attention-logit
path (f = x @ (W@a), leaky-relu bias, exp input) stays fp32 since the
softmax is sensitive to absolute errors in the logits.

kernel(**inputs) takes the full unsharded inputs and returns the full
[4096, 256] output.
"""

from contextlib import ExitStack

import numpy as np
import ml_dtypes

import concourse.mybir as mybir
import concourse.tile as tile
from concourse import bacc
from concourse.bass_utils import run_bass_kernel_spmd

P = 128
N_NODES = 4096
S = 512                    # nodes per core
NB = N_NODES // P          # 32 global key blocks
H = 4
LAYERS = [(512, 512), (2048, 512), (2048, 64)]
F32 = mybir.dt.float32
BF16 = mybir.dt.bfloat16
AF = mybir.ActivationFunctionType
ALU = mybir.AluOpType

_CACHE = {}


def _build():
    nc = bacc.Bacc("TRN2", target_bir_lowering=False, debug=False, num_devices=8)

    xT0_d = nc.dram_tensor("xT0", [512, S], F32, kind="ExternalInput")
    adjT_d = nc.dram_tensor("adjT", [N_NODES, S], BF16, kind="ExternalInput")
    W_d = []
    WA_d = []
    for li, (fin, fout) in enumerate(LAYERS):
        W_d.append(nc.dram_tensor(f"W{li}", [H, fin, fout], BF16, kind="ExternalInput"))
        WA_d.append(nc.dram_tensor(f"WA{li}", [fin, 2 * H], F32, kind="ExternalInput"))
    outT_d = nc.dram_tensor("outT", [H * 64, S], F32, kind="ExternalOutput")

    with tile.TileContext(nc) as tc:
        with ExitStack() as ctx:
            constp = ctx.enter_context(tc.tile_pool(name="const", bufs=1))
            adjp = ctx.enter_context(tc.tile_pool(name="adjp", bufs=1))
            xtp = ctx.enter_context(tc.tile_pool(name="xt", bufs=20))
            xbp = ctx.enter_context(tc.tile_pool(name="xb", bufs=21))
            wtp = ctx.enter_context(tc.tile_pool(name="wt", bufs=17))
            wap = ctx.enter_context(tc.tile_pool(name="wap", bufs=17))
            hfp = ctx.enter_context(tc.tile_pool(name="hfp", bufs=3))
            hgp = ctx.enter_context(tc.tile_pool(name="hgp", bufs=4))
            scp = ctx.enter_context(tc.tile_pool(name="scp", bufs=2))
            fbp = ctx.enter_context(tc.tile_pool(name="fbp", bufs=1))
            fdp = ctx.enter_context(tc.tile_pool(name="fdp", bufs=2))
            frp = ctx.enter_context(tc.tile_pool(name="frp", bufs=3))
            evp = ctx.enter_context(tc.tile_pool(name="evp", bufs=3))
            rcp = ctx.enter_context(tc.tile_pool(name="rcp", bufs=2))
            psA = ctx.enter_context(tc.tile_pool(name="psA", bufs=2, space="PSUM"))
            psO = ctx.enter_context(tc.tile_pool(name="psO", bufs=4, space="PSUM"))
            psR = ctx.enter_context(tc.tile_pool(name="psR", bufs=1, space="PSUM"))
            dr = ctx.enter_context(tc.tile_pool(name="dram", bufs=1, space="DRAM"))

            ones_r = constp.tile([1, P], F32, tag="ones_r")
            nc.any.memset(ones_r[:], 1.0)
            ones_c = constp.tile([P, 1], BF16, tag="ones_c")
            nc.any.memset(ones_c[:], 1.0)

            # resident adjacency (transposed slab), bf16, [key m, own query n]
            adjT_res = adjp.tile([P, NB, S], BF16, tag="adjT")
            nc.sync.dma_start(
                adjT_res[:], adjT_d[:].rearrange("(nb p) n -> p nb n", p=P)
            )

            # layer-0 x^T (own nodes): f32 for the f matmuls + bf16 for h
            xt_cur = []
            xb_cur = []
            for kb in range(4):
                t = xtp.tile([P, S], F32, tag="xt")
                nc.sync.dma_start(t[:], xT0_d[kb * P:(kb + 1) * P, :])
                xt_cur.append(t)
                tb = xbp.tile([P, S], BF16, tag="xb")
                nc.vector.tensor_copy(tb[:], t[:])
                xb_cur.append(tb)

            for li, (fin, fout) in enumerate(LAYERS):
                KB = fin // P
                agh_in = dr.tile([H, S, fout], BF16, tag=f"aghi{li}")
                agh_out = [
                    dr.tile([8, S, fout], BF16, tag=f"agho{li}_{hh}",
                            name=f"agho{li}_{hh}", addr_space="Shared")
                    for hh in range(H)
                ]
                agf_in = dr.tile([2 * H, S], F32, tag=f"agfi{li}")
                agf_out = dr.tile([8, 2 * H, S], F32, tag=f"agfo{li}",
                                  addr_space="Shared")

                # ---- phase A1: f = x @ WA (fp32) ----
                wa_tiles = []
                for kb in range(KB):
                    t = wap.tile([P, 2 * H], F32, tag="wa")
                    nc.sync.dma_start(t[:], WA_d[li][kb * P:(kb + 1) * P, :])
                    wa_tiles.append(t)
                for b in range(4):
                    pf = psA.tile([P, S], F32, tag="ph", name=f"pf{li}_{b}")
                    for kb in range(KB):
                        nc.tensor.matmul(
                            pf[:, 0:2 * H],
                            xt_cur[kb][:, b * P:(b + 1) * P],
                            wa_tiles[kb][:],
                            start=(kb == 0), stop=(kb == KB - 1),
                        )
                    f_sb = hfp.tile([P, 2 * H], F32, tag="fsb")
                    nc.vector.tensor_copy(f_sb[:], pf[:, 0:2 * H])
                    for j in range(2 * H):
                        nc.sync.dma_start(
                            agf_in[j, b * P:(b + 1) * P], f_sb[:, j:j + 1]
                        )

                # f gather is tiny; issue it before the h matmuls so it hides
                nc.gpsimd.collective_compute(
                    "AllGather", ALU.bypass,
                    replica_groups=[list(range(8))],
                    ins=[agf_in[:].opt()], outs=[agf_out[:].opt()],
                )

                # ---- phase A2: h = x @ W (bf16), shared-weight loop ----
                w_tiles = {}
                for h in range(H):
                    for kb in range(KB):
                        t = wtp.tile([P, fout], BF16, tag="wt", name=f"w{li}_{h}_{kb}")
                        nc.sync.dma_start(t[:], W_d[li][h, kb * P:(kb + 1) * P, :])
                        w_tiles[(h, kb)] = t
                for h in range(H):
                    for b in range(4):
                        ph = psA.tile([P, S], F32, tag="ph", name=f"ph{li}_{h}_{b}")
                        for kb in range(KB):
                            nc.tensor.matmul(
                                ph[:, 0:fout],
                                xb_cur[kb][:, b * P:(b + 1) * P],
                                w_tiles[(h, kb)][:],
                                start=(kb == 0), stop=(kb == KB - 1),
                            )
                        h_sb = hfp.tile([P, fout], BF16, tag="hsb")
                        nc.vector.tensor_copy(h_sb[:], ph[:, 0:fout])
                        nc.gpsimd.dma_start(agh_in[h, b * P:(b + 1) * P, :], h_sb[:])
                    # per-head gather overlaps the next head's h matmuls
                    nc.gpsimd.collective_compute(
                        "AllGather", ALU.bypass,
                        replica_groups=[list(range(8))],
                        ins=[agh_in[h].opt()], outs=[agh_out[h][:].opt()],
                    )

                # ---- phase C: f_src broadcast + f_dst layout ----
                fsb_bcast = fbp.tile([P, H, S], F32, tag="fsb_b")
                for h in range(H):
                    fr = frp.tile([1, S], F32, tag="fr")
                    nc.sync.dma_start(fr[:], agf_in[2 * h:2 * h + 1, :])
                    pb = psA.tile([P, S], F32, tag="ph", name=f"pb{li}_{h}")
                    nc.tensor.matmul(pb[:], ones_r[:], fr[:], start=True, stop=True)
                    nc.scalar.copy(fsb_bcast[:, h, :], pb[:])
                fdst_sb = fdp.tile([P, H, NB], F32, tag="fdst")
                for h in range(H):
                    for r in range(8):
                        nc.sync.dma_start(
                            fdst_sb[:, h, r * 4:(r + 1) * 4],
                            agf_out[r, 2 * h + 1, :].rearrange("(mh ml) -> ml mh", ml=P),
                        )

                # ---- phase D: attention ----
                xt_next = []
                xb_next = []
                nob = 1 if fout == 64 else 4
                pending_evict = None
                for h in range(H):
                    po = [
                        psO.tile([P, S], F32, tag="po", name=f"po{li}_{h}_{ob}")
                        for ob in range(nob)
                    ]
                    prs = psR.tile([1, S], F32, tag="prs", name=f"prs{li}_{h}")
                    for mbg in range(NB // 4):
                        lr4 = scp.tile([P, 4, S], F32, tag="lr")
                        for i in range(4):
                            nc.scalar.activation(
                                lr4[:, i, :], fsb_bcast[:, h, :], AF.Prelu,
                                bias=fdst_sb[:, h, 4 * mbg + i:4 * mbg + i + 1],
                                scale=1.0, alpha=0.2,
                            )
                        ex4 = scp.tile([P, 4, S], BF16, tag="ex")
                        nc.scalar.activation(ex4[:], lr4[:], AF.Exp, bias=0.0, scale=1.0)
                        st4 = scp.tile([P, 4, S], BF16, tag="st")
                        nc.vector.tensor_tensor(
                            st4[:], ex4[:], adjT_res[:, 4 * mbg:4 * mbg + 4, :], ALU.mult
                        )
                        for i in range(4):
                            mb = 4 * mbg + i
                            r, bsub = mb // 4, mb % 4
                            s_t = st4[:, i, :]
                            hg = hgp.tile([P, fout], BF16, tag="hg")
                            nc.gpsimd.dma_start(
                                hg[:], agh_out[h][r, bsub * P:(bsub + 1) * P, :]
                            )
                            for ob in range(nob):
                                nc.tensor.matmul(
                                    po[ob][:, :] if fout != 64 else po[ob][0:64, :],
                                    hg[:, ob * P:(ob + 1) * P] if fout != 64 else hg[:],
                                    s_t,
                                    start=(mb == 0), stop=(mb == NB - 1),
                                )
                            nc.tensor.matmul(
                                prs[:], ones_c[:], s_t,
                                start=(mb == 0), stop=(mb == NB - 1),
                            )

                    # free the PSUM banks promptly (ACT copies), defer the
                    # DVE normalize+elu so the next head's score TTs are not
                    # queued behind a premature PE wait on the vector engine
                    rows = 64 if fout == 64 else P
                    praw = []
                    for ob in range(nob):
                        src = po[ob][0:64, :] if fout == 64 else po[ob][:]
                        pr_sb = evp.tile([rows, S], F32, tag="praw")
                        nc.scalar.copy(pr_sb[:], src)
                        praw.append(pr_sb)
                    rsum = rcp.tile([1, S], F32, tag="rsum")
                    nc.scalar.copy(rsum[:], prs[:])
                    pb2 = psA.tile([P, S], F32, tag="ph", name=f"pb2{li}_{h}")
                    nc.tensor.matmul(pb2[:], ones_r[:], rsum[:], start=True, stop=True)
                    rb0 = rcp.tile([P, S], F32, tag="rb0")
                    nc.scalar.copy(rb0[:], pb2[:])

                    def _evict(praw=praw, rb0=rb0, li=li, rows=rows):
                        rb = rcp.tile([P, S], F32, tag="rb", name=f"rb{li}")
                        nc.vector.reciprocal_approx_fast(rb[:], rb0[:])
                        for pr_sb in praw:
                            t0 = evp.tile([rows, S], F32, tag="t0", name=f"t0{li}")
                            nc.vector.tensor_tensor(t0[:], pr_sb[:], rb[0:rows, :], ALU.mult)
                            # elu(x) = min(exp(x) - 1, relu(x))
                            em = evp.tile([rows, S], F32, tag="em", name=f"em{li}")
                            nc.scalar.activation(em[:], t0[:], AF.Exp, bias=0.0, scale=1.0)
                            rl = evp.tile([rows, S], F32, tag="rl", name=f"rl{li}")
                            nc.vector.tensor_scalar_max(rl[:], t0[:], 0.0)
                            xnt = xtp.tile([rows, S], F32, tag="xt", name=f"xt{li}")
                            nc.vector.scalar_tensor_tensor(
                                xnt[:], em[:], -1.0, rl[:], ALU.add, ALU.min
                            )
                            if li == 2:
                                em2 = evp.tile([rows, S], F32, tag="em", name=f"em2{li}")
                                nc.scalar.activation(em2[:], xnt[:], AF.Exp, bias=0.0, scale=1.0)
                                rl2 = evp.tile([rows, S], F32, tag="rl", name=f"rl2{li}")
                                nc.vector.tensor_scalar_max(rl2[:], xnt[:], 0.0)
                                x2 = xtp.tile([rows, S], F32, tag="xt", name=f"x2{li}")
                                nc.vector.scalar_tensor_tensor(
                                    x2[:], em2[:], -1.0, rl2[:], ALU.add, ALU.min
                                )
                                xnt = x2
                            xt_next.append(xnt)
                            if li < 2:
                                xbn = xbp.tile([rows, S], BF16, tag="xb", name=f"xb{li}")
                                nc.vector.tensor_copy(xbn[:], xnt[:])
                                xb_next.append(xbn)

                    if pending_evict is not None:
                        pending_evict()
                    pending_evict = _evict

                if pending_evict is not None:
                    pending_evict()
                    pending_evict = None

                xt_cur = xt_next
                xb_cur = xb_next

            # final output: xt_cur is 4 tiles of [64, 512] (head-major)
            for h in range(H):
                nc.sync.dma_start(outT_d[h * 64:(h + 1) * 64, :], xt_cur[h][:])

    nc.compile()
    return nc


def build_in_maps(inputs):
    node_feats = np.ascontiguousarray(inputs["node_feats"], dtype=np.float32)
    adj = np.asarray(inputs["adj"], dtype=np.float32)
    Ws = [np.asarray(inputs[f"W{i}"], dtype=np.float32) for i in range(3)]
    As = [np.asarray(inputs[f"a{i}"], dtype=np.float32) for i in range(3)]

    WAs = []
    for W, a in zip(Ws, As):
        wa = np.einsum(
            "hfo,hjo->fhj", W.astype(np.float64), a.astype(np.float64)
        ).reshape(W.shape[1], 2 * H).astype(np.float32)
        WAs.append(np.ascontiguousarray(wa))
    Wbf = [W.astype(ml_dtypes.bfloat16) for W in Ws]

    in_maps = []
    for c in range(8):
        rows = slice(c * S, (c + 1) * S)
        m = {
            "xT0": np.ascontiguousarray(node_feats[rows].T),
            "adjT": np.ascontiguousarray(adj[rows].T).astype(ml_dtypes.bfloat16),
        }
        for i in range(3):
            m[f"W{i}"] = Wbf[i]
            m[f"WA{i}"] = WAs[i]
        in_maps.append(m)
    return in_maps


def kernel(**inputs):
    if "nc" not in _CACHE:
        _CACHE["nc"] = _build()
    nc = _CACHE["nc"]
    in_maps = build_in_maps(inputs)
    res = run_bass_kernel_spmd(nc, in_maps, core_ids=list(range(8)))
    out = np.concatenate([r["outT"].T for r in res.results], axis=0)
    return np.ascontiguousarray(out, dtype=np.float32)


if __name__ == "__main__":
    rng = np.random.default_rng(0)
    fake = {
        "node_feats": rng.standard_normal((N_NODES, 512), dtype=np.float32),
        "edge_feats": rng.standard_normal((131072, 16), dtype=np.float32),
        "edge_indices": rng.integers(0, N_NODES, (2, 131072)).astype(np.int32),
        "adj": np.maximum(
            (rng.random((N_NODES, N_NODES)) < 0.01).astype(np.float32),
            np.eye(N_NODES, dtype=np.float32),
        ),
    }
    for i, (fin, fout) in enumerate(LAYERS):
        fake[f"W{i}"] = (rng.standard_normal((H, fin, fout)) * 0.05).astype(np.float32)
        fake[f"a{i}"] = (rng.standard_normal((H, 2, fout)) * 0.05).astype(np.float32)
    o = kernel(**fake)
    print("kernel output", o.shape, o.dtype, np.abs(o).mean())
